# revision 22
# baseline (speedup 1.0000x reference)
"""NNUE HalfKP EmbeddingBag + MLP kernel for 8 Trainium2 NeuronCores.

Strategy (data-parallel over the batch):
  - 16384 bags are split into 8 row-balanced shards of ~2048 consecutive bags
    (boundaries at gathered-row quantiles, so every core moves the same ~61k
    rows; the embedding table is replicated per core in DRAM, bf16).
  - Each core's rows are packed into two dense gather STREAMS:
      L (table idx < 32768) and H (idx >= 32768, rebased) -- int16 gather
      indices can't address all 41024 rows, so the two streams use different
      DRAM base pointers.  Streams are fetched with big 4096-row
      gpsimd.dma_gather chunks (32 matmul tiles each) into an SBUF ring,
      amortizing the ~1us fixed SWDGE cost per gather.
  - A core's bags are grouped into 17 blocks of <=128 consecutive bags with
    block boundaries at L-stream row quantiles, so the per-block stream tile
    ranges line up across cores (the SPMD program unions them).
  - Segment-sum into per-block PSUM tiles with TensorE matmuls against 0/1
    selection matrices built on DVE via tensor_scalar(is_equal) (per-partition
    scalar ptr keeps operands packed so the fast 2x DVE mode applies).
    A row-tile that straddles a block boundary is matmul'd once per block it
    touches, with complementary selection columns (-1 bag => zero column).
  - bias1 + relu are applied AFTER the [bag,h]->[h,bag] transpose on the
    (otherwise idle) Activation engine using its per-partition bias operand,
    so no bias rows are gathered at all.
  - The tiny MLP (fc2 -> relu -> out) runs in bf16 on-chip; each core writes
    17*128 output slots which the host maps back to its actual bags.
"""

import numpy as np

import concourse.bacc as bacc
import concourse.mybir as mybir
from concourse.tile import TileContext
from concourse.masks import make_identity

# ---------------- problem constants (hardcoded per spec) ----------------
NUM_FEATURES = 41024
HIDDEN = 256
FC2 = 32
BATCH = 16384
N_IDX = 491520
N_CORES = 8

BAGS_PER_CORE = BATCH // N_CORES       # nominal (cores hold ~this many bags)
BLOCK_BAGS = 128                       # max bags per PSUM block
NBLK = 17                              # blocks per core (<=128 bags each)
OUT_COLS = NBLK * BLOCK_BAGS           # 2176 output slots per core
SPLIT = 32768                          # int16 index limit
TILE = 128                             # rows per matmul tile
CHUNK_ROWS = 1024                      # rows per dma_gather (HW cap)
TPC = CHUNK_ROWS // TILE               # 32 tiles per gather chunk
N_QUEUES = 4
NRING = 14                             # gather dst ring buffers
PREFETCH_BLOCKS = 2                    # issue gathers this many blocks early

TABLE_BF16 = True                      # gather the table in bf16


def _ceil_div(a, b):
    return -(-a // b)


def _host_prep(indices, offsets):
    """Build per-core gather streams, per-(block,tile) selection columns,
    valid counts, and the shared schedule."""
    indices = np.asarray(indices).astype(np.int64)
    offsets = np.asarray(offsets).astype(np.int64)
    n = indices.shape[0]
    seg = np.clip(
        np.searchsorted(offsets, np.arange(n), side="right") - 1, 0, BATCH - 1
    )
    sizes = np.bincount(seg, minlength=BATCH)
    cum = np.concatenate([[0], np.cumsum(sizes)])     # rows before bag g

    # row-balanced core boundaries (bag granularity)
    B = [0]
    for c in range(1, N_CORES):
        t = int(np.searchsorted(cum, c * n // N_CORES))
        t = max(B[-1] + 1, min(t, BATCH - (N_CORES - c)))
        B.append(t)
    B.append(BATCH)

    streams = []   # per core: Li,Lb,Hi,Hb, lL,lH (row offsets per block), bnd
    for c in range(N_CORES):
        g0, g1 = B[c], B[c + 1]
        nbags = g1 - g0
        assert nbags <= OUT_COLS
        lo, hi = cum[g0], cum[g1]
        idx_c = indices[lo:hi]
        bag_c = seg[lo:hi] - g0                        # 0..nbags-1, nondecr.
        low = idx_c < SPLIT
        Li, Lb = idx_c[low], bag_c[low]
        Hi, Hb = idx_c[~low] - SPLIT, bag_c[~low]
        cumL = np.concatenate([[0], np.cumsum(np.bincount(Lb, minlength=nbags))])

        # 17 block boundaries (bag indices) at L-row quantiles, <=128 bags each
        bnd = [0]
        for b in range(1, NBLK):
            tgt = int(np.searchsorted(cumL, b * len(Li) // NBLK))
            lo_b = max(bnd[-1], nbags - (NBLK - b) * BLOCK_BAGS)
            hi_b = min(bnd[-1] + BLOCK_BAGS, nbags)
            bnd.append(max(lo_b, min(tgt, hi_b)))
        bnd.append(nbags)
        assert all(
            0 <= bnd[b + 1] - bnd[b] <= BLOCK_BAGS for b in range(NBLK)
        ), f"core {c} block sizes {np.diff(bnd)}"

        cumH = np.concatenate([[0], np.cumsum(np.bincount(Hb, minlength=nbags))])
        bnd = np.asarray(bnd)
        streams.append(dict(Li=Li, Lb=Lb, Hi=Hi, Hb=Hb,
                            lL=cumL[bnd], lH=cumH[bnd],
                            bnd=bnd, g0=g0, nbags=nbags))

    TL = max(_ceil_div(max(len(s["Li"]), 1), TILE) for s in streams)
    TH = max(_ceil_div(max(len(s["Hi"]), 1), TILE) for s in streams)
    NGL = _ceil_div(TL, TPC)
    NGH = _ceil_div(TH, TPC)
    n_gathers = NGL + NGH
    total_tiles = TL + TH
    idx_cols = n_gathers * (CHUNK_ROWS // 16)

    # shared per-block tile ranges (union over cores); tile ids: L 0..TL-1,
    # H TL..TL+TH-1
    pairs = []           # (block, tile, col)
    tiles_of_block = []  # [block] -> list of (tile, col)
    col = 0
    for b in range(NBLK):
        tl = []
        ls = min(s["lL"][b] // TILE for s in streams)
        le = min(max(_ceil_div(s["lL"][b + 1], TILE) for s in streams), TL)
        for t in range(ls, max(le, ls)):
            tl.append((t, col)); pairs.append((b, t, col)); col += 1
        hs = min(s["lH"][b] // TILE for s in streams)
        he = min(max(_ceil_div(s["lH"][b + 1], TILE) for s in streams), TH)
        for t0 in range(hs, max(he, hs)):
            t = TL + t0
            tl.append((t, col)); pairs.append((b, t, col)); col += 1
        assert tl, f"block {b} has no tiles"
        tiles_of_block.append(tl)
    n_pairs = col

    def chunk_of(t):
        return t // TPC if t < TL else NGL + (t - TL) // TPC

    # gather issue schedule: first-use order, issued PREFETCH_BLOCKS blocks
    # early, but never overwriting a ring slot whose previous chunk still has
    # un-emitted consumers
    first_use, last_use = {}, {}
    for (b, t, _c) in pairs:
        ch = chunk_of(t)
        first_use.setdefault(ch, b)
        last_use[ch] = b
    order = sorted(first_use, key=lambda ch: (first_use[ch], ch))
    for ch in range(n_gathers):
        if ch not in first_use:           # unused chunk: issue at the very end
            order.append(ch)
            first_use[ch] = NBLK - 1
            last_use[ch] = NBLK - 1
    issue_at = []
    for j, ch in enumerate(order):
        at = max(0, first_use[ch] - PREFETCH_BLOCKS)
        if j >= NRING:
            at = max(at, last_use[order[j - NRING]] + 1)
        if j > 0:
            at = max(at, issue_at[j - 1])
        assert at <= first_use[ch], (
            f"gather ring too small: chunk {ch} must issue at block {at} "
            f"but is first used at block {first_use[ch]}"
        )
        issue_at.append(at)

    # ---- per-core blobs ----
    idx_blobs, bag_blobs, metas = [], [], []
    for c in range(N_CORES):
        s = streams[c]
        idx_arr = np.zeros((128, idx_cols), dtype=np.int16)
        bag_arr = np.full((128, n_pairs), -1.0, dtype=np.float32)
        meta = np.zeros((1, n_gathers), dtype=np.int32)

        for ch in range(n_gathers):
            if ch < NGL:
                vals, r0 = s["Li"], ch * CHUNK_ROWS
            else:
                vals, r0 = s["Hi"], (ch - NGL) * CHUNK_ROWS
            chunk_idx = np.full(CHUNK_ROWS, -1, dtype=np.int64)
            nvalid = min(max(len(vals) - r0, 0), CHUNK_ROWS)
            if nvalid > 0:
                chunk_idx[:nvalid] = vals[r0 : r0 + nvalid]
            else:
                chunk_idx[0] = 0
                nvalid = 1
            meta[0, ch] = nvalid
            # idx wrap: row i -> [i%16, i//16], replicated to 128 partitions
            w = chunk_idx.reshape(CHUNK_ROWS // 16, 16).T.astype(np.int16)
            icol = ch * (CHUNK_ROWS // 16)
            idx_arr[:, icol : icol + CHUNK_ROWS // 16] = np.tile(w, (8, 1))

        # selection columns: block id and block-relative bag per stream row
        for (stream_tiles, bags, bounds, toff) in (
            (TL, s["Lb"], s["lL"], 0),
            (TH, s["Hb"], s["lH"], TL),
        ):
            nrow = stream_tiles * TILE
            rowblk = np.full(nrow, -1, dtype=np.int64)
            rowbag = np.full(nrow, -1.0, dtype=np.float64)
            m = len(bags)
            if m > 0:
                rb = np.searchsorted(bounds, np.arange(m), side="right") - 1
                rb = np.clip(rb, 0, NBLK - 1)
                rowblk[:m] = rb
                rowbag[:m] = bags - s["bnd"][rb]
            for (b, t, cc) in pairs:
                if toff <= t < toff + stream_tiles:
                    rr = (t - toff) * TILE
                    blk = rowblk[rr : rr + TILE]
                    bag_arr[:, cc] = np.where(
                        blk == b, rowbag[rr : rr + TILE], -1.0
                    ).astype(np.float32)

        idx_blobs.append(idx_arr); bag_blobs.append(bag_arr); metas.append(meta)

    full_chunk = [
        all(int(metas[c][0, ch]) == CHUNK_ROWS for c in range(N_CORES))
        for ch in range(n_gathers)
    ]

    # output slot -> global bag mapping (for host-side unshard)
    slotmaps = []
    for c in range(N_CORES):
        s = streams[c]
        slots = np.full(s["nbags"], -1, dtype=np.int64)
        for b in range(NBLK):
            g0b, g1b = int(s["bnd"][b]), int(s["bnd"][b + 1])
            slots[g0b:g1b] = b * BLOCK_BAGS + np.arange(g1b - g0b)
        assert (slots >= 0).all()
        slotmaps.append((int(s["g0"]), int(s["nbags"]), slots))

    sched = dict(
        pairs=pairs, tiles_of_block=tiles_of_block,
        order=order, issue_at=issue_at,
        NGL=NGL, NGH=NGH, n_pairs=n_pairs, slotmaps=slotmaps,
        full_chunk=full_chunk,
    )
    return sched, TL, TH, total_tiles, idx_cols, n_gathers, idx_blobs, bag_blobs, metas


def _build_program(sched, TL, TH, total_tiles, idx_cols, n_gathers, reps=1):
    import os as _os
    STRIP = _os.environ.get("KSTRIP", "")
    tdt = mybir.dt.bfloat16 if TABLE_BF16 else mybir.dt.float32
    f32 = mybir.dt.float32
    NGL = sched["NGL"]
    n_pairs = sched["n_pairs"]
    tiles_of_block = sched["tiles_of_block"]
    order, issue_at = sched["order"], sched["issue_at"]

    nc = bacc.Bacc(
        "TRN2", dynamic_dma_scratch_size=65536, num_swdge_queues=N_QUEUES
    )
    table = nc.dram_tensor(
        "table", [NUM_FEATURES, HIDDEN], tdt, kind="ExternalInput"
    )
    idx_d = nc.dram_tensor("idxs", [128, idx_cols], mybir.dt.int16, kind="ExternalInput")
    bag_d = nc.dram_tensor("bags", [128, n_pairs], f32, kind="ExternalInput")
    meta_d = nc.dram_tensor("meta", [1, n_gathers], mybir.dt.int32, kind="ExternalInput")
    iota_d = nc.dram_tensor("iota", [128, 128], tdt, kind="ExternalInput")
    b1_d = nc.dram_tensor("b1", [128, 2], f32, kind="ExternalInput")       # bias1
    w2_d = nc.dram_tensor("w2", [HIDDEN, FC2], tdt, kind="ExternalInput")  # fc2_w.T
    b2_d = nc.dram_tensor("b2", [FC2, 1], f32, kind="ExternalInput")
    w3_d = nc.dram_tensor("w3", [FC2, 1], tdt, kind="ExternalInput")       # out_w.T
    b3_d = nc.dram_tensor("b3", [1, 1], f32, kind="ExternalInput")
    out_d = nc.dram_tensor("out", [1, OUT_COLS], f32, kind="ExternalOutput")

    def chunk_of(t):
        return t // TPC if t < TL else NGL + (t - TL) // TPC

    def chunk_first_tile(ch):
        return ch * TPC if ch < NGL else TL + (ch - NGL) * TPC

    # MLP column groups (<=512 wide for the f32 PSUM bank)
    mlp_bounds = list(range(0, OUT_COLS, 512)) + [OUT_COLS]
    mlp_bounds = sorted(set(min(x, OUT_COLS) for x in mlp_bounds))

    with TileContext(nc) as tc_:
        with (
            tc_.tile_pool(name="const", bufs=1) as cpool,
            tc_.tile_pool(name="seg", bufs=16) as spool,
            tc_.tile_pool(name="hraw", bufs=3) as hpool,
            tc_.tile_pool(name="h2", bufs=2) as h2pool,
            tc_.tile_pool(name="ph", bufs=2, space="PSUM") as phpool,
            tc_.tile_pool(name="pt", bufs=2, space="PSUM") as ptpool,
            tc_.tile_pool(name="pm", bufs=2, space="PSUM") as pmpool,
        ):
            idx_sb = cpool.tile([128, idx_cols], mybir.dt.int16)
            bag_sb = cpool.tile([128, n_pairs], f32)
            meta_sb = cpool.tile([1, n_gathers], mybir.dt.int32)
            iota_sb = cpool.tile([128, 128], tdt)
            b1_sb = cpool.tile([128, 2], f32)
            w2_sb = cpool.tile([128, 2 * FC2], tdt)
            b2_sb = cpool.tile([FC2, 1], f32)
            w3_sb = cpool.tile([FC2, 1], tdt)
            b3_sb = cpool.tile([1, 1], f32)
            ident = cpool.tile([128, 128], tdt)
            hT = cpool.tile([128, 2 * OUT_COLS], tdt)
            out_sb = cpool.tile([1, OUT_COLS], f32)
            if STRIP:
                nc.vector.memset(out_sb[:, :], 0.0)

            nc.sync.dma_start(idx_sb[:, :], idx_d[:, :])
            nc.sync.dma_start(bag_sb[:, :], bag_d[:, :])
            nc.sync.dma_start(meta_sb[:, :], meta_d[:, :])
            nc.sync.dma_start(iota_sb[:, :], iota_d[:, :])
            nc.sync.dma_start(b1_sb[:, :], b1_d[:, :])
            nc.sync.dma_start(w2_sb[:, 0:FC2], w2_d[0:128, :])
            nc.sync.dma_start(w2_sb[:, FC2 : 2 * FC2], w2_d[128:256, :])
            nc.sync.dma_start(b2_sb[:, :], b2_d[:, :])
            nc.sync.dma_start(w3_sb[:, :], w3_d[:, :])
            nc.sync.dma_start(b3_sb[:, :], b3_d[:, :])
            make_identity(nc, ident[:, :])

            # gather-dst ring, zero-seeded so pad tiles never feed NaN into
            # the zero-weighted matmul columns
            dst_ring = []
            for i in range(NRING):
                t = cpool.tile([128, TPC * HIDDEN], tdt, name=f"dstr{i}", bufs=1)
                nc.vector.memset(t[:, :], 0.0)
                dst_ring.append(t)

            regs = [nc.gpsimd.alloc_register(f"nv{i}") for i in range(4)]
            gather_no = {"n": 0}
            slot_of_chunk = {}

            def issue_gather(ch):
                gno = gather_no["n"]
                slot = gno % NRING
                slot_of_chunk[ch] = slot
                dst = dst_ring[slot]
                src = table[0:SPLIT, :] if ch < NGL else table[SPLIT:, :]
                if sched["full_chunk"][ch]:
                    reg = CHUNK_ROWS
                else:
                    reg = regs[gno % 4]
                    nc.gpsimd.reg_load(reg, meta_sb[0:1, ch : ch + 1])
                icol = ch * (CHUNK_ROWS // 16)
                nc.gpsimd.dma_gather(
                    dst[:, :].rearrange("p (t e) -> p t e", e=HIDDEN),
                    src,
                    idx_sb[:, icol : icol + CHUNK_ROWS // 16],
                    CHUNK_ROWS,
                    reg,
                    HIDDEN,
                    queue_num=gno % N_QUEUES,
                )
                gather_no["n"] = gno + 1

            def emit_tail(b, psum):
                hraw = hpool.tile([128, HIDDEN], tdt, name="hraw", tag="hraw")
                nc.scalar.activation(
                    hraw[:, :], psum[:, :], mybir.ActivationFunctionType.Copy
                )
                for half in range(2):
                    pt = ptpool.tile([128, 128], tdt, name="ptt", tag="ptt")
                    nc.tensor.transpose(
                        pt[:, :],
                        hraw[:, half * 128 : (half + 1) * 128],
                        ident[:, :],
                    )
                    nc.scalar.activation(
                        hT[
                            :,
                            half * OUT_COLS
                            + b * 128 : half * OUT_COLS
                            + (b + 1) * 128,
                        ],
                        pt[:, :],
                        mybir.ActivationFunctionType.Relu,
                        bias=b1_sb[:, half : half + 1],
                    )

            def emit_mlp_group(g):
                c0, c1 = mlp_bounds[g], mlp_bounds[g + 1]
                p2 = pmpool.tile([FC2, 512], f32, name="p2t", tag="p2")
                for half in range(2):
                    nc.tensor.matmul(
                        p2[:, 0 : c1 - c0],
                        lhsT=w2_sb[:, half * FC2 : (half + 1) * FC2],
                        rhs=hT[:, half * OUT_COLS + c0 : half * OUT_COLS + c1],
                        start=(half == 0),
                        stop=(half == 1),
                    )
                h2 = h2pool.tile([FC2, 512], tdt, name="h2t", tag="h2")
                nc.scalar.activation(
                    h2[:, 0 : c1 - c0], p2[:, 0 : c1 - c0],
                    mybir.ActivationFunctionType.Relu,
                    bias=b2_sb[:, :],
                )
                p3 = pmpool.tile([1, 512], f32, name="p3t", tag="p3")
                nc.tensor.matmul(
                    p3[:, 0 : c1 - c0], lhsT=w3_sb[:, :], rhs=h2[:, 0 : c1 - c0],
                    start=True, stop=True,
                )
                nc.vector.tensor_scalar_add(
                    out_sb[:, c0:c1], p3[:, 0 : c1 - c0], b3_sb[0:1, 0:1]
                )

            def one_pass():
                gptr = {"j": 0}
                pending = []
                state = {"tails": 0, "mlp": 0}

                def emit_tail_and_mlp(b, psum):
                    emit_tail(b, psum)
                    state["tails"] += 1
                    while (STRIP == "" and state["mlp"] < len(mlp_bounds) - 1
                           and mlp_bounds[state["mlp"] + 1]
                           <= state["tails"] * BLOCK_BAGS):
                        emit_mlp_group(state["mlp"])
                        state["mlp"] += 1
                for b in range(NBLK):
                    if STRIP != "segsonly":
                        while gptr["j"] < len(order) and issue_at[gptr["j"]] <= b:
                            issue_gather(order[gptr["j"]])
                            gptr["j"] += 1
                    if STRIP == "gathers":
                        continue
                    tl = tiles_of_block[b]
                    psum = phpool.tile([128, HIDDEN], f32, name="psumh", tag="psumh")
                    for j, (t, col) in enumerate(tl):
                        seg = spool.tile([128, TILE], tdt, name="segt", tag="seg")
                        nc.vector.tensor_scalar(
                            out=seg[:, :],
                            in0=iota_sb[:, :],
                            scalar1=bag_sb[:, col : col + 1],
                            scalar2=None,
                            op0=mybir.AluOpType.is_equal,
                        )
                        if STRIP in ("segs", "segsonly"):
                            continue
                        ch = chunk_of(t)
                        toff = t - chunk_first_tile(ch)
                        dst = dst_ring[slot_of_chunk[ch]]
                        nc.tensor.matmul(
                            psum[:, :],
                            lhsT=seg[:, :],
                            rhs=dst[:, toff * HIDDEN : (toff + 1) * HIDDEN],
                            start=(j == 0),
                            stop=(j == len(tl) - 1),
                        )
                    if STRIP in ("segs", "segsonly"):
                        continue
                    pending.append((b, psum))
                    if len(pending) > 1:
                        emit_tail_and_mlp(*pending.pop(0))

                while pending:
                    emit_tail_and_mlp(*pending.pop(0))

            for _rep in range(reps):
                one_pass()
            nc.sync.dma_start(out_d[:, :], out_sb[:, :])
    nc.compile()
    return nc


def _make_in_maps(inputs, sched_data):
    (sched, TL, TH, total_tiles, idx_cols, n_gathers,
     idx_blobs, bag_blobs, metas) = sched_data
    import ml_dtypes
    tdt_np = ml_dtypes.bfloat16 if TABLE_BF16 else np.float32
    embed_weight = np.asarray(inputs["embed_weight"], dtype=np.float32).astype(tdt_np)
    bias1 = np.asarray(inputs["bias1"], dtype=np.float32)
    fc2_w = np.asarray(inputs["fc2_w"], dtype=np.float32)
    fc2_b = np.asarray(inputs["fc2_b"], dtype=np.float32)
    out_w = np.asarray(inputs["out_w"], dtype=np.float32)
    out_b = np.asarray(inputs["out_b"], dtype=np.float32)
    iota = np.broadcast_to(
        np.arange(128, dtype=np.float32)[None, :], (128, 128)
    ).astype(tdt_np).copy()
    common = {
        "table": embed_weight,
        "iota": iota,
        "b1": bias1.reshape(2, 128).T.copy(),
        "w2": fc2_w.T.astype(tdt_np).copy(),
        "b2": fc2_b.reshape(FC2, 1),
        "w3": out_w.reshape(1, FC2).T.astype(tdt_np).copy(),
        "b3": out_b.reshape(1, 1),
    }
    in_maps = []
    for c in range(N_CORES):
        m = dict(common)
        m["idxs"] = idx_blobs[c]
        m["bags"] = bag_blobs[c]
        m["meta"] = metas[c]
        in_maps.append(m)
    return in_maps


def unshard_outputs(sched, outs):
    """outs: per-core arrays of OUT_COLS floats -> full [BATCH] output."""
    full = np.zeros(BATCH, dtype=np.float32)
    for c, (g0, nbags, slots) in enumerate(sched["slotmaps"]):
        full[g0 : g0 + nbags] = np.asarray(outs[c]).reshape(OUT_COLS)[slots]
    return full


def kernel(**inputs) -> np.ndarray:
    from concourse.bass_utils import run_bass_kernel_spmd

    sched_data = _host_prep(inputs["indices"], inputs["offsets"])
    sched, TL, TH, total_tiles, idx_cols, n_gathers = sched_data[:6]
    nc = _build_program(sched, TL, TH, total_tiles, idx_cols, n_gathers)
    in_maps = _make_in_maps(inputs, sched_data)
    res = run_bass_kernel_spmd(nc, in_maps, core_ids=list(range(N_CORES)))
    return unshard_outputs(
        sched, [res.results[c]["out"] for c in range(N_CORES)]
    )
